# revision 49
# baseline (speedup 1.0000x reference)
"""DeepSeek MLA head — Trainium2 Bass kernel, 8 NeuronCores.

Sharding: 8 cores = 2 batches x 4 cores. Core c: batch b=c//4, j=c%4 owns
heads [4j,4j+4), token supertile j (512 tokens) for latent compute, and hid
output columns [512j,512j+512) for o_proj.

Phases:
  P1a  each core computes q/kv low-rank latents + RMSNorm + shared k_pe rope
       for ITS supertile only (1/4 of the replicated baseline work).
  AG   two AllGathers (q latents, kv latents) within each 4-core batch group
       via DRAM bounce buffers exchange the normalized latents.
  P1b  per-head up-projections (q_b + rope, k_nope, V) over all 4 supertiles
       from the gathered latents. k_nope pairs 2 heads per matmul (M=128).
  P2   attention per head: scores / exp / AV. exp batched over 2-chunk
       [128,1024] PSUM groups; softmax denominators accumulated with DVE f16
       adds + one ones-matmul per (head, q-supertile). exp is shifted by -6
       (softmax shift-invariant) so f16 partial sums cannot overflow.
  P3   per q-supertile: AllGather the 4 local heads' attention outputs so
       every core sees all 16 heads, then compute o_proj for this core's
       512 hid columns with the full 16-head contraction. Output is an exact
       [2048, 512] f32 slice; the host concatenates (no reduction).

Layout: activations transposed [feature, token] on-chip; matmul contractions
land on the partition axis. Host folds RMSNorm gains + DeepSeek RoPE
interleave permutation into the weights. Matmul operands f16, f32 PSUM.
"""
import sys
import types

sys.path.insert(0, "/opt/trn_rl_repo")

import numpy as np

B, S, HID, NH = 2, 2048, 2048, 16
ROPE, NOPE, VDIM = 64, 64, 128
QHEAD, QLORA, KVLORA = 128, 682, 256
THETA = 128000.0
SCALE = 1.0 / float(np.sqrt(128.0))
EPS = 1e-6
NEGC = -6.0          # exp shift: exp(SCALE*s - 6), softmax-invariant
HPC = 4              # heads per core
NCORES = 8
QCH = [128, 128, 128, 128, 128, 42]   # qlora partition chunks
QW = [256, 256, 170]                  # qlora rows per chunk-pair
NST = 4              # 512-token supertiles per batch
STW = 512
GROUPS = [[0, 1, 2, 3], [4, 5, 6, 7]]

_PROGRAM = None


def _ensure_axon_hooks_shim():
    if "antenv.axon_hooks" in sys.modules:
        return
    try:
        from trn_agent_boot.trn_boot import _ntff_profile_via_ctypes
        hook = _ntff_profile_via_ctypes("/opt/axon/libaxon_pjrt.so")
    except Exception:
        hook = None
    m = types.ModuleType("antenv.axon_hooks")
    m.get_axon_ntff_profile_hook = lambda: hook
    m.set_axon_ntff_profile_hook = lambda h: None
    sys.modules["antenv.axon_hooks"] = m


def _build_program():
    import concourse.bass as bass  # noqa: F401
    import concourse.mybir as mybir
    import concourse.tile as tile
    from concourse import bacc

    f16 = mybir.dt.float16
    f32 = mybir.dt.float32
    AF = mybir.ActivationFunctionType

    nc = bacc.Bacc("TRN2", target_bir_lowering=False, debug=False,
                   num_devices=NCORES)
    # const APs for activation bias args
    for cv in (EPS, NEGC):
        t = nc.alloc_sbuf_tensor(f"const-{cv}", [128, 1], f32)
        nc.gpsimd.memset(t.ap(), cv)
        nc.const_aps.aps[(f32, cv)] = t.ap()
    nc.all_engine_barrier()

    def din(name, shape, dt=f16):
        return nc.dram_tensor(name, shape, dt, kind="ExternalInput").ap()

    xT = din("xT", [HID, STW])            # my supertile of x, transposed
    waq = din("waq", [HID, QLORA])        # q_a_w
    wakv = din("wakv", [HID, 384])        # kv_a_w cols: [ckv 256|0s 64|kpe-perm 64]
    wqb = din("wqb", [QLORA, HPC * 128])  # my heads: [nope64|pe64-perm], ln folded
    wkn = din("wkn", [KVLORA, HPC * 64])  # my heads: knope cols, ln folded
    wv = din("wv", [KVLORA, HPC * 128])   # my heads: v cols, ln folded
    wo = din("wo", [NH * VDIM, STW])      # ALL heads' o_w rows x my hid cols
    cosT = din("cosT", [128, S])          # rows 0:64 = 1, rows 64:128 = cos
    sinT = din("sinT", [128, S])          # rows 0:64 = 0, rows 64:128 = sin
    cosM = din("cosM", [128, STW])        # my supertile slice
    sinM = din("sinM", [128, STW])
    rotT = din("rotT", [128, 128])        # transposed rotate-half matrix
    maskT = din("maskT", [128, 896])      # shifted causal window: m[k,c]=(k<=c-384)
    out = nc.dram_tensor("out", [S, STW], f32, kind="ExternalOutput").ap()

    LATQW = 6 * STW          # 3072: q latent cols in gather payload
    LATKW = 3 * STW          # 1536: ckv0, ckv1, kpe

    from contextlib import ExitStack
    with tile.TileContext(nc) as tc, ExitStack() as ctx:
        const = ctx.enter_context(tc.tile_pool(name="const", bufs=1))
        waqp = ctx.enter_context(tc.tile_pool(name="waqp", bufs=16))
        wakvp = ctx.enter_context(tc.tile_pool(name="wakvp", bufs=16))
        xtwo = ctx.enter_context(tc.tile_pool(name="xtwo", bufs=16))
        latqp = ctx.enter_context(tc.tile_pool(name="latqp", bufs=6))
        latkp = ctx.enter_context(tc.tile_pool(name="latkp", bufs=6))
        rawp = ctx.enter_context(tc.tile_pool(name="rawp", bufs=1))
        sqp = ctx.enter_context(tc.tile_pool(name="sqp", bufs=1))
        smallp = ctx.enter_context(tc.tile_pool(name="smallp", bufs=2))
        bcp = ctx.enter_context(tc.tile_pool(name="bcp", bufs=2))
        scr1 = ctx.enter_context(tc.tile_pool(name="scr1", bufs=1))
        persist = ctx.enter_context(tc.tile_pool(name="persist", bufs=HPC))
        ptp = ctx.enter_context(tc.tile_pool(name="ptp", bufs=2))
        accp = ctx.enter_context(tc.tile_pool(name="accp", bufs=2))
        oep = ctx.enter_context(tc.tile_pool(name="oep", bufs=1))
        dramp = ctx.enter_context(tc.tile_pool(name="dram", bufs=1, space="DRAM"))
        # PSUM: 8 banks total = big 2x[128,1024] (4) + out 3x[128,512] (3)
        #       + misc 1x[128,512] (1)
        ps_big = ctx.enter_context(tc.tile_pool(name="ps_big", bufs=2, space="PSUM"))
        ps_out = ctx.enter_context(tc.tile_pool(name="ps_out", bufs=3, space="PSUM"))
        ps_misc = ctx.enter_context(tc.tile_pool(name="ps_misc", bufs=1, space="PSUM"))

        # ---- constants into SBUF ----
        sb_waq = [waqp.tile([128, QLORA], f16, tag="waq", name=f"waq{hc}")
                  for hc in range(16)]
        sb_wakv = [wakvp.tile([128, 384], f16, tag="wakv", name=f"wakv{hc}")
                   for hc in range(16)]
        sb_wqb = const.tile([128, 6 * HPC * 128], f16, tag="wqb")
        sb_wkn = const.tile([128, 2 * HPC * 64], f16, tag="wkn")
        sb_wv = const.tile([128, 2 * HPC * 128], f16, tag="wv")
        sb_cos = const.tile([128, S], f16, tag="cos")
        sb_sin = const.tile([128, S], f16, tag="sin")
        sb_cosM = const.tile([128, STW], f16, tag="cosM")
        sb_sinM = const.tile([128, STW], f16, tag="sinM")
        sb_rot = const.tile([128, 128], f16, tag="rot")
        sb_mask = const.tile([128, 896], f16, tag="mask")
        sb_ones = const.tile([128, 1], f16, tag="ones")
        sb_onesr = const.tile([1, 128], f16, tag="onesr")

        qoff = [0, 128, 256, 384, 512, 640]
        W = HPC * 128
        xt = [xtwo.tile([128, STW], f16, tag="xt", name=f"xt{hc}")
              for hc in range(16)]
        # Load order = first-use order: kv latents run first, then q
        for hc in range(16):
            nc.sync.dma_start(out=xt[hc][:], in_=xT[hc * 128:(hc + 1) * 128, :])
            nc.sync.dma_start(out=sb_wakv[hc][:], in_=wakv[hc * 128:(hc + 1) * 128, :])
        nc.sync.dma_start(out=sb_rot[:], in_=rotT[:])
        nc.sync.dma_start(out=sb_cosM[:], in_=cosM[:])
        nc.sync.dma_start(out=sb_sinM[:], in_=sinM[:])
        for c in range(2):
            nc.sync.dma_start(out=sb_wkn[:, c * HPC * 64:(c + 1) * HPC * 64],
                              in_=wkn[c * 128:(c + 1) * 128, :])
            nc.sync.dma_start(out=sb_wv[:, c * W:(c + 1) * W],
                              in_=wv[c * 128:(c + 1) * 128, :])
        for pcol in range(3):
            cl, cr = 256 * pcol, min(QLORA, 256 * (pcol + 1))
            for hc in range(16):
                nc.sync.dma_start(out=sb_waq[hc][:, cl:cr],
                                  in_=waq[hc * 128:(hc + 1) * 128, cl:cr])
        for c in range(6):
            nc.sync.dma_start(out=sb_wqb[:QCH[c], c * W:(c + 1) * W],
                              in_=wqb[qoff[c]:qoff[c] + QCH[c], :])
        nc.sync.dma_start(out=sb_cos[:], in_=cosT[:])
        nc.sync.dma_start(out=sb_sin[:], in_=sinT[:])
        nc.sync.dma_start(out=sb_mask[:], in_=maskT[:])
        nc.vector.memset(sb_ones[:], 1.0)
        nc.vector.memset(sb_onesr[:], 1.0)

        # persistent per-head tensors
        qfT = [persist.tile([128, S], f16, tag="qf", name=f"qfT{h}") for h in range(HPC)]
        kfT = [persist.tile([128, S], f16, tag="kf", name=f"kfT{h}") for h in range(HPC)]
        VT = [persist.tile([128, 16 * VDIM], f16, tag="vh", name=f"VT{h}") for h in range(HPC)]
        aout = [persist.tile([128, S], f16, tag="aout", name=f"aout{h}") for h in range(HPC)]

        # ================= P1a: latents for MY supertile =================
        # q latents are gathered UN-normalized (rstd folded into the rope
        # cos/sin tables per supertile after the gather) so the bounce DMAs
        # and the first collective can launch as early as possible.
        sums = ps_out.tile([64, STW], f32, tag="out", name="sums")  # row0 q, row32 k

        # latent gathers: GKV (kraw+kpe) first, then GQ split in two
        bgq_in = dramp.tile([128, 3584], f16, name="bgq_in", tag="bgq_in")
        bgq_out = dramp.tile([4 * 128, 3584], f16, name="bgq_out", tag="bgq_out")
        bgk_in = dramp.tile([128, 1536], f16, name="bgk_in", tag="bgk_in")
        bgk_out = dramp.tile([4 * 128, 1536], f16, name="bgk_out", tag="bgk_out")

        # kv latents: 1 ckv chunk-pair + kpe chunk, then GKV gather
        psk = ps_big.tile([128, 1024], f32, tag="big", name="klat")
        for half in range(2):
            for hc in range(16):
                nc.tensor.matmul(
                    psk[:, half * STW:(half + 1) * STW],
                    sb_wakv[hc][:, half * 128:(half + 1) * 128],
                    xt[hc][:],
                    start=(hc == 0), stop=(hc == 15))
        kraw = rawp.tile([128, 1024], f16, tag="rawk")
        nc.vector.tensor_copy(kraw[:], psk[:])
        sqk = sqp.tile([128, 1024], f16, tag="sq")
        nc.scalar.activation(sqk[:], psk[:], AF.Square)
        for half in range(2):
            nc.tensor.matmul(sums[32:33, :], sb_ones[:, :],
                             sqk[:, half * STW:(half + 1) * STW],
                             start=(half == 0), stop=(half == 1))

        psp = ps_misc.tile([128, STW], f32, tag="misc", name="kpelat")
        for hc in range(16):
            nc.tensor.matmul(psp[:], sb_wakv[hc][:, 256:384], xt[hc][:],
                             start=(hc == 0), stop=(hc == 15))
        kperaw = rawp.tile([128, STW], f16, tag="kpe")
        nc.vector.tensor_copy(kperaw[:], psp[:])

        # rstd for k, normalize kraw
        stdk = smallp.tile([1, STW], f32, tag="stdk", bufs=1)
        nc.scalar.activation(stdk[:], sums[32:33, :], AF.Sqrt,
                             bias=EPS, scale=1.0 / KVLORA)
        rstdkf = smallp.tile([1, STW], f32, tag="rstdkf", bufs=1)
        nc.vector.reciprocal_approx_fast(out=rstdkf[:], in_=stdk[:])
        rstdk = smallp.tile([1, STW], f16, tag="rstdk", bufs=1)
        nc.vector.tensor_copy(rstdk[:], rstdkf[:])
        bck = ps_misc.tile([128, STW], f32, tag="misc", name="bck")
        nc.tensor.matmul(bck[:], sb_onesr[:], rstdk[:1, :], start=True, stop=True)
        bcks = bcp.tile([128, STW], f16, tag="bc", name="bcks")
        nc.vector.tensor_copy(bcks[:], bck[:])
        nc.vector.tensor_mul(kraw[:, 0:STW], kraw[:, 0:STW], bcks[:])
        nc.vector.tensor_mul(kraw[:, STW:1024], kraw[:, STW:1024], bcks[:])

        # shared k_pe rope (rows 64:128 hold the permuted k_pe)
        rps = ps_misc.tile([128, STW], f32, tag="misc", name="rps")
        nc.tensor.matmul(rps[:], sb_rot[:], kperaw[:], start=True, stop=True)
        t1k = scr1.tile([128, STW], f16, tag="t1k")
        nc.vector.tensor_mul(t1k[:], rps[:], sb_sinM[:])
        t2k = scr1.tile([128, STW], f16, tag="t2k")
        nc.vector.tensor_mul(t2k[:], kperaw[:], sb_cosM[:])
        kpero = rawp.tile([128, STW], f16, tag="kpeo")
        nc.vector.tensor_add(kpero[:], t1k[:], t2k[:])

        nc.gpsimd.dma_start(bgk_in[:, 0:1024], kraw[:])
        nc.gpsimd.dma_start(bgk_in[:, 1024:1536], kpero[:])
        nc.gpsimd.collective_compute(
            "AllGather", mybir.AluOpType.bypass, replica_groups=GROUPS,
            ins=[bgk_in.opt()], outs=[bgk_out.opt()])

        # q latents: 3 chunk-pairs, gathered raw (rstd folded in post-gather)
        qraw = []
        for p in range(3):
            ps = ps_big.tile([128, 1024], f32, tag="big", name=f"qlat{p}")
            for half in range(2):
                c = 2 * p + half
                for hc in range(16):
                    nc.tensor.matmul(
                        ps[:QCH[c], half * STW:(half + 1) * STW],
                        sb_waq[hc][:, qoff[c]:qoff[c] + QCH[c]],
                        xt[hc][:],
                        start=(hc == 0), stop=(hc == 15))
            raw = rawp.tile([128, 1024], f16, tag="rawq", name=f"rawq{p}", bufs=2)
            nc.vector.tensor_copy(raw[:], ps[:])
            sq = sqp.tile([128, 1024], f16, tag="sq")
            nc.scalar.activation(sq[:], ps[:], AF.Square)
            for half in range(2):
                c = 2 * p + half
                nc.tensor.matmul(sums[0:1, :], sb_ones[:QCH[c], :],
                                 sq[:QCH[c], half * STW:(half + 1) * STW],
                                 start=(c == 0), stop=(c == 5))
            nc.gpsimd.dma_start(bgq_in[:, p * 1024:(p + 1) * 1024], raw[:])
            qraw.append(raw)

        # rstd for q, then GQ gather
        stdq = smallp.tile([1, STW], f32, tag="stdq", bufs=1)
        nc.scalar.activation(stdq[:], sums[0:1, :], AF.Sqrt,
                             bias=EPS, scale=1.0 / QLORA)
        rstdqf = smallp.tile([1, STW], f32, tag="rstdqf", bufs=1)
        nc.vector.reciprocal_approx_fast(out=rstdqf[:], in_=stdq[:])
        rstdq = smallp.tile([1, STW], f16, tag="rstdq", bufs=1)
        nc.vector.tensor_copy(rstdq[:], rstdqf[:])
        nc.gpsimd.dma_start(bgq_in[0:1, 3072:3584], rstdq[:])
        nc.gpsimd.collective_compute(
            "AllGather", mybir.AluOpType.bypass, replica_groups=GROUPS,
            ins=[bgq_in.opt()], outs=[bgq_out.opt()])

        # ================= P1b: per-head projections over all supertiles ====
        # kn/V pass (needs only GKV, which lands first)
        gks = []
        for st in range(NST):
            cols = slice(st * STW, (st + 1) * STW)
            gk = [latkp.tile([128, STW], f16, tag="latk", name=f"gk{st}_{c}",
                             bufs=6)
                  for c in range(3)]
            for c in range(3):
                nc.gpsimd.dma_start(
                    gk[c][:],
                    bgk_out[st * 128:(st + 1) * 128,
                            c * STW:(c + 1) * STW])
            gks.append(gk)
            for hp in range(2):
                pskn = ps_out.tile([128, STW], f32, tag="out", name=f"kn{st}_{hp}")
                for c in range(2):
                    nc.tensor.matmul(
                        pskn[:],
                        sb_wkn[:, c * HPC * 64 + hp * 128:c * HPC * 64 + (hp + 1) * 128],
                        gk[c][:],
                        start=(c == 0), stop=(c == 1))
                nc.scalar.copy(out=kfT[2 * hp][0:64, cols], in_=pskn[0:64, :])
                nc.scalar.copy(out=kfT[2 * hp + 1][0:64, cols], in_=pskn[64:128, :])
            for h in range(HPC):
                nc.vector.tensor_copy(kfT[h][64:128, cols], gk[2][64:128, :])
            for h in range(HPC):
                psv = ps_out.tile([128, STW], f32, tag="out", name=f"psv{st}_{h}")
                for tcn in range(4):
                    for c in range(2):
                        nc.tensor.matmul(
                            psv[:, tcn * VDIM:(tcn + 1) * VDIM],
                            gk[c][:, tcn * 128:(tcn + 1) * 128],
                            sb_wv[:, c * W + h * 128:c * W + (h + 1) * 128],
                            start=(c == 0), stop=(c == 1))
                nc.vector.tensor_copy(VT[h][:, st * STW:(st + 1) * STW], psv[:])

        # q_b + rope for one supertile (fused into the P2 loop below)
        def qb_block(st):
            cols = slice(st * STW, (st + 1) * STW)
            gq = []
            for c in range(6):
                t = latqp.tile([128, STW], f16, tag="latq", name=f"gq{st}_{c}",
                               bufs=12)
                nc.gpsimd.dma_start(
                    t[:QCH[c], :],
                    bgq_out[st * 128:st * 128 + QCH[c],
                            c * STW:(c + 1) * STW])
                gq.append(t)
            grs = latkp.tile([1, STW], f16, tag="lrs", name=f"grs{st}", bufs=1)
            nc.gpsimd.dma_start(grs[:],
                                bgq_out[st * 128:st * 128 + 1, 3072:3584])

            bcq = ps_misc.tile([128, STW], f32, tag="misc", name=f"bcq{st}")
            nc.tensor.matmul(bcq[:], sb_onesr[:], grs[:1, :], start=True, stop=True)
            bcqs = bcp.tile([128, STW], f16, tag="bc", name=f"bcqs{st}")
            nc.scalar.copy(out=bcqs[:], in_=bcq[:])
            sinq = scr1.tile([128, STW], f16, tag="sinq", name=f"sinq{st}", bufs=2)
            cosq = scr1.tile([128, STW], f16, tag="cosq", name=f"cosq{st}", bufs=2)
            nc.vector.tensor_mul(sinq[:], sb_sin[:, cols], bcqs[:])
            nc.vector.tensor_mul(cosq[:], sb_cos[:, cols], bcqs[:])

            for pr in range(2):
                qra = scr1.tile([128, 2 * STW], f16, tag="qra",
                                name=f"qra{st}_{pr}", bufs=2)
                for i in range(2):
                    h = 2 * pr + i
                    psq = ps_out.tile([128, STW], f32, tag="out",
                                      name=f"psq{st}_{h}")
                    for c in range(6):
                        nc.tensor.matmul(
                            psq[:],
                            sb_wqb[:QCH[c], c * W + h * 128:c * W + (h + 1) * 128],
                            gq[c][:QCH[c], :],
                            start=(c == 0), stop=(c == 5))
                    nc.vector.tensor_copy(qra[:, i * STW:(i + 1) * STW], psq[:])
                rq = ps_big.tile([128, 1024], f32, tag="big", name=f"rq{st}_{pr}")
                nc.tensor.matmul(rq[:, 0:STW], sb_rot[:], qra[:, 0:STW],
                                 start=True, stop=True)
                nc.tensor.matmul(rq[:, STW:1024], sb_rot[:], qra[:, STW:1024],
                                 start=True, stop=True)
                t1q = scr1.tile([128, 2 * STW], f16, tag="t1q",
                                name=f"t1q{st}_{pr}", bufs=2)
                for i in range(2):
                    h = 2 * pr + i
                    hs = slice(i * STW, (i + 1) * STW)
                    nc.vector.tensor_mul(t1q[:, hs], rq[:, hs], sinq[:])
                    # t2 in place over qra (rot matmul already consumed it)
                    nc.vector.tensor_mul(qra[:, hs], qra[:, hs], cosq[:])
                    nc.vector.tensor_add(qfT[h][:, cols],
                                         t1q[:, hs], qra[:, hs])

        # ================= P2 / P3, interleaved per q-supertile =============
        # pre-zero the pt rotation slots: diagonal chunks write partial
        # widths, and the mask-mul must see finite values in skipped cols
        for z in range(2):
            ptz = ptp.tile([128, 1024], f16, tag="pt", name=f"ptz{z}")
            nc.vector.memset(ptz[:], 0.0)

        def norm(qs, qcols, h, ssums, accs):
            bca = ps_misc.tile([128, STW], f32, tag="misc", name=f"bca{qs}_{h}")
            nc.tensor.matmul(bca[:], sb_onesr[:], ssums[h][:1, :],
                             start=True, stop=True)
            bcas = bcp.tile([128, STW], f16, tag="bc", name=f"bcas{qs}_{h}")
            nc.vector.tensor_copy(bcas[:], bca[:])
            nc.vector.tensor_mul(aout[h][:, qcols], accs[h][:], bcas[:])

        def p2_block(qs):
            qcols = slice(qs * STW, (qs + 1) * STW)
            nkc = 4 * qs + 4
            ng = nkc // 2
            accs, ssums = [], []
            # diagonal chunks first (for qs>0) so the accumulation start
            # lands on the full-width j=0 chunk and the stop on a full-width
            # off-diagonal chunk, letting the partial-width diagonal AV
            # matmuls skip fully-masked columns.
            kcs = (list(range(4 * qs, nkc)) + list(range(0, 4 * qs))
                   if qs > 0 else list(range(nkc)))
            for h in range(HPC):
                outT = ps_out.tile([128, STW], f32, tag="out", name=f"oT{qs}_{h}")
                acc = accp.tile([128, STW], f16, tag="acc")

                stps, pts = {}, {}
                def sc(g):
                    stp = ps_big.tile([128, 1024], f32, tag="big",
                                      name=f"sc{qs}_{h}_{g}")
                    trim = [0, 0]
                    for half in range(2):
                        kc = kcs[2 * g + half]
                        j = kc - 4 * qs
                        # columns q < 128*j of a diagonal chunk are fully
                        # masked: skip them in the scores matmul and the exp
                        tr = 128 * j if j > 0 else 0
                        trim[half] = tr
                        nc.tensor.matmul(
                            stp[:, half * STW + tr:(half + 1) * STW],
                            kfT[h][:, kc * 128:(kc + 1) * 128],
                            qfT[h][:, qs * STW + tr:(qs + 1) * STW],
                            start=True, stop=True)
                    pt = ptp.tile([128, 1024], f16, tag="pt")
                    if trim[0] == 0 and trim[1] == 0:
                        nc.scalar.activation(pt[:], stp[:], AF.Exp,
                                             bias=NEGC, scale=SCALE)
                    else:
                        for half in range(2):
                            tr = trim[half]
                            nc.scalar.activation(
                                pt[:, half * STW + tr:(half + 1) * STW],
                                stp[:, half * STW + tr:(half + 1) * STW],
                                AF.Exp, bias=NEGC, scale=SCALE)
                    for half in range(2):
                        kc = kcs[2 * g + half]
                        j = kc - 4 * qs
                        if j >= 0:
                            ph = pt[:, half * STW:(half + 1) * STW]
                            nc.vector.tensor_mul(
                                ph, ph, sb_mask[:, 384 - 128 * j:896 - 128 * j])
                    pts[g] = pt

                def av(g, first):
                    pt = pts.pop(g)
                    for half in range(2):
                        idx = 2 * g + half
                        kc = kcs[idx]
                        j = kc - 4 * qs
                        tr = 128 * j if (j > 0 and qs > 0) else 0
                        ph = pt[:, half * STW + tr:(half + 1) * STW]
                        nc.tensor.matmul(outT[:, tr:STW],
                                         VT[h][:, kc * VDIM:(kc + 1) * VDIM],
                                         ph,
                                         start=(idx == 0), stop=(idx == nkc - 1))
                        phf = pt[:, half * STW:(half + 1) * STW]
                        if idx == 1:
                            nc.vector.tensor_add(acc[:], first[0], phf)
                        elif idx == 0:
                            first[0] = phf
                        else:
                            nc.vector.tensor_add(acc[:], acc[:], phf)

                first = [None]
                sc(0)
                # normalize the previous head here: its 1/sum is ready and
                # the PE was just fed by sc(0), so the bca matmul can't stall
                if h >= 1:
                    norm(qs, qcols, h - 1, ssums, accs)
                for g in range(ng):
                    if g + 1 < ng:
                        sc(g + 1)
                    av(g, first)
                ssum = ps_misc.tile([1, STW], f32, tag="misc", name=f"ss{qs}_{h}")
                nc.tensor.matmul(ssum[:], sb_ones[:, :], acc[:],
                                 start=True, stop=True)
                rsf = smallp.tile([1, STW], f32, tag="rsf", bufs=2)
                nc.vector.reciprocal_approx_fast(out=rsf[:], in_=ssum[:])
                rs = smallp.tile([1, STW], f16, tag="rs", bufs=4)
                nc.vector.tensor_copy(rs[:], rsf[:])
                ssums.append(rs)
                accs.append(outT)
            norm(qs, qcols, HPC - 1, ssums, accs)

        def p3_comm(qs):
            qcols = slice(qs * STW, (qs + 1) * STW)
            ba_in = dramp.tile([128, 4 * STW], f16, tag="ba_in",
                               name=f"bain{qs}", bufs=2)
            ba_out = dramp.tile([4 * 128, 4 * STW], f16, tag="ba_out",
                                name=f"baout{qs}", bufs=2)
            for h in range(HPC):
                nc.gpsimd.dma_start(ba_in[:, h * STW:(h + 1) * STW],
                                    aout[h][:, qcols])
            nc.gpsimd.collective_compute(
                "AllGather", mybir.AluOpType.bypass, replica_groups=GROUPS,
                ins=[ba_in.opt()], outs=[ba_out.opt()])
            return ba_out

        def p3_load(qs, ba_out):
            # aoG tiles reuse dead pool slots; even/odd supertiles use
            # disjoint pools so load(qs+1) can prefetch during block(qs)
            aoG = []
            for hh in range(16):
                if qs % 2 == 0:
                    t = waqp.tile([128, STW], f16, tag="waq",
                                  name=f"aoG{qs}_{hh}")
                elif hh < 12:
                    t = latqp.tile([128, STW], f16, tag="latq",
                                   name=f"aoG{qs}_{hh}", bufs=12)
                else:
                    t = latkp.tile([128, STW], f16, tag="latk",
                                   name=f"aoG{qs}_{hh}", bufs=6)
                nc.sync.dma_start(
                    out=t[:],
                    in_=ba_out[(hh // 4) * 128:(hh // 4 + 1) * 128,
                               (hh % 4) * STW:(hh % 4 + 1) * STW])
                aoG.append(t)
            return aoG

        def p3_block(qs, aoG):
            for tcn in range(4):
                pso = ps_out.tile([128, STW], f32, tag="out", name=f"pso{qs}_{tcn}")
                for hh in range(16):
                    nc.tensor.matmul(
                        pso[:],
                        aoG[hh][:, tcn * 128:(tcn + 1) * 128],
                        sb_wo[hh][:],
                        start=(hh == 0), stop=(hh == 15))
                ob = oep.tile([128, STW], f32, tag="oe")
                nc.scalar.copy(out=ob[:], in_=pso[:])
                nc.sync.dma_start(
                    out=out[qs * STW + tcn * 128:qs * STW + (tcn + 1) * 128, :],
                    in_=ob[:])

        # o_w loads into the xT stream slots (xT dead after P1a); issued
        # after the latent gathers so they don't compete with collective DMA
        sb_wo = []
        for hh in range(16):
            t = xtwo.tile([128, STW], f16, tag="xt", name=f"wo{hh}")
            nc.sync.dma_start(out=t[:], in_=wo[hh * 128:(hh + 1) * 128, :])
            sb_wo.append(t)

        for st in range(NST):
            qb_block(st)
        bas = []
        for qs in range(NST):
            p2_block(qs)
            bas.append(p3_comm(qs))
        for qs in range(NST):
            aoG = p3_load(qs, bas[qs])
            p3_block(qs, aoG)

    nc.compile()
    return nc


def _host_prep(inputs):
    f16 = np.float16
    x = np.asarray(inputs["x"], np.float32)
    q_a_w = np.asarray(inputs["q_a_w"], np.float32)
    q_a_ln = np.asarray(inputs["q_a_ln_w"], np.float32)
    q_b_w = np.asarray(inputs["q_b_w"], np.float32)
    kv_a_w = np.asarray(inputs["kv_a_w"], np.float32)
    kv_a_ln = np.asarray(inputs["kv_a_ln_w"], np.float32)
    kv_b_w = np.asarray(inputs["kv_b_w"], np.float32)
    o_w = np.asarray(inputs["o_w"], np.float32)

    perm = np.concatenate([np.arange(0, ROPE, 2), np.arange(1, ROPE, 2)])
    q_b_f = q_b_w * q_a_ln[:, None]
    kv_b_f = kv_b_w * kv_a_ln[:, None]

    # kv_a padded: [ckv 256 | zeros 64 | kpe perm 64]
    wakv = np.concatenate(
        [kv_a_w[:, :KVLORA],
         np.zeros((HID, 64), np.float32),
         kv_a_w[:, KVLORA:][:, perm]], axis=1).astype(f16)
    waq = q_a_w.astype(f16)

    # rope tables (transposed [dim, pos])
    inv = 1.0 / (THETA ** (np.arange(0, ROPE, 2, dtype=np.float64) / ROPE))
    freqs = np.outer(np.arange(S, dtype=np.float64), inv)      # [S, 32]
    cos64 = np.concatenate([np.cos(freqs), np.cos(freqs)], -1).T  # [64, S]
    sin64 = np.concatenate([np.sin(freqs), np.sin(freqs)], -1).T
    cosT = np.concatenate([np.ones((64, S)), cos64], 0).astype(f16)
    sinT = np.concatenate([np.zeros((64, S)), sin64], 0).astype(f16)

    # rotate-half matrix: out = ROT @ xp, nonzero only on rows/cols 64:128
    R64 = np.zeros((64, 64), np.float32)
    for j in range(32):
        R64[j, 32 + j] = -1.0
        R64[32 + j, j] = 1.0
    ROT = np.zeros((128, 128), np.float32)
    ROT[64:, 64:] = R64
    rotT = ROT.T.astype(f16)

    # shifted causal window: maskT[k, c] = k <= c - 384; slice j is
    # cols [384-128j, 896-128j) giving mask_j[k, q] = k <= q - 128*j
    k_i = np.arange(128)[:, None]
    c_i = np.arange(896)[None, :]
    maskT = (k_i <= c_i - 384).astype(f16)

    in_maps = []
    for core in range(NCORES):
        b = core // 4
        j = core % 4
        heads = [HPC * j + i for i in range(HPC)]
        wqb = np.concatenate(
            [np.concatenate(
                [q_b_f[:, h * QHEAD:h * QHEAD + NOPE],
                 q_b_f[:, h * QHEAD + NOPE:(h + 1) * QHEAD][:, perm]], 1)
             for h in heads], axis=1).astype(f16)
        wkn = np.concatenate(
            [kv_b_f[:, h * (NOPE + VDIM):h * (NOPE + VDIM) + NOPE]
             for h in heads], axis=1).astype(f16)
        wv = np.concatenate(
            [kv_b_f[:, h * (NOPE + VDIM) + NOPE:(h + 1) * (NOPE + VDIM)]
             for h in heads], axis=1).astype(f16)
        wo = o_w[:, j * STW:(j + 1) * STW].astype(f16)   # all heads' rows
        scols = slice(j * STW, (j + 1) * STW)
        in_maps.append({
            "xT": np.ascontiguousarray(x[b].T[:, scols]).astype(f16),
            "waq": waq, "wakv": wakv, "wqb": wqb, "wkn": wkn, "wv": wv,
            "wo": wo, "cosT": cosT, "sinT": sinT,
            "cosM": np.ascontiguousarray(cosT[:, scols]),
            "sinM": np.ascontiguousarray(sinT[:, scols]),
            "rotT": rotT, "maskT": maskT,
        })
    return in_maps


def kernel(**inputs):
    global _PROGRAM
    _ensure_axon_hooks_shim()
    from concourse.bass_utils import run_bass_kernel_spmd

    if _PROGRAM is None:
        _PROGRAM = _build_program()
    in_maps = _host_prep(inputs)
    res = run_bass_kernel_spmd(_PROGRAM, in_maps, list(range(NCORES)))
    out = np.zeros((B, S, HID), np.float32)
    for core in range(NCORES):
        b, j = core // 4, core % 4
        out[b][:, j * STW:(j + 1) * STW] = res.results[core]["out"]
    return out
